# revision 8
# baseline (speedup 1.0000x reference)
"""Trainium2 Bass kernel for MultiHeadedAttention + residual + LayerNorm.

Problem: B=2, S=2048, D=1024, H=16 heads (DK=64), fp32 in/out.
  q,k,v = (x @ W + b) per projection; per-head scaled-dot-product attention
  with full S x S score matrix; out = LayerNorm(attn_out + query) * gamma + beta.

Sharding (8 NeuronCores, tensor-parallel over heads):
  Core c owns heads {2c, 2c+1} == output channels [128c, 128c+128).

fp8 design (final tolerance is 2e-2 relative-to-absmax; attention output is
~30x smaller than the residual, so the attention path tolerates ~5% noise):
  - All matmul operands are fp8e4 (TRN e4m3, max +-240). Weights are
    prescaled by 32 on the host so W*32 ~ N(0,1) avoids fp8 subnormals; the
    1/32 is folded into the PSUM->SBUF copy after each projection.
  - q/k projections (channel-major qT/kT = W.T @ xT) and the v projection
    (token-major v = x @ W, via stationary=xT / moving=W) run as fp8
    DoubleRow matmuls: two 128-row K-tiles per instruction.
  - bq/bk fold into the projection copy's scalar add; bv folds into the
    host-prepared residual (softmax rows sum to 1).
  - Scores sT = kT.T @ qT per head (K=64, two heads at PE row quadrants
    0/64). exp on ACT with scale=1/8 and bias=-ln(16): fp8e4 output range
    tops out ~15.3 (e4m3 overflows to Inf above 240, so headroom matters);
    the 1/16 cancels in the softmax ratio.
  - PV: out[65, q] = [v | 1].T @ pt as fp8 DoubleRow over kpos-tile pairs;
    the ones column accumulates the softmax denominator in PSUM row 64.
  - PE-transpose (bf16) back to token-major, divide by denominator, add
    fp32 residual, bn_stats per 128-channel slice, per-batch 16KB AllReduce
    of (mean, E[y^2]) partials, Newton rsqrt on DVE (no ACT table thrash),
    normalize, write token-major fp32 output slices.
Host assembles the 8 channel slices into the full (2, 2048, 1024) output.
"""

import numpy as np

B, S, D, H, DK = 2, 2048, 1024, 16, 64
T = B * S              # 4096 flattened tokens
NCORES = 8
NCH = D // NCORES      # 128 channels (2 heads) per core
KT = D // 128          # 8 contraction tiles for projections
NTILE = T // 128       # 32 token tiles of 128
ST = S // 128          # 16 key tiles per batch
TQ = S // 512          # 4 query chunks of 512 per batch

LN32 = 3.4657359027997265  # ln(32): exp scale keeps fp8e4 pt <= ~128 (max s/8 = 8.37)

_COMPILED = None


def _build_program(with_collective: bool = True, repeat: int = 1,
                   dr_proj: bool = True, dr_pv: bool = True,
                   debug_taps: bool = False):
    import concourse.bass as bass
    import concourse.mybir as mybir
    import concourse.tile as tile
    from concourse import bacc
    from concourse.masks import make_identity

    F32 = mybir.dt.float32
    BF16 = mybir.dt.bfloat16
    FP8 = mybir.dt.float8e4
    AF = mybir.ActivationFunctionType
    DR = mybir.MatmulPerfMode.DoubleRow
    MULT = mybir.AluOpType.mult

    nc = bacc.Bacc(
        "TRN2",
        target_bir_lowering=False,
        debug=False,
        enable_asserts=False,
        num_devices=NCORES,
    )

    xqT_d = nc.dram_tensor("xqT", (D, T), BF16, kind="ExternalInput")
    xkT_d = nc.dram_tensor("xkT", (D, T), BF16, kind="ExternalInput")
    xvT_d = nc.dram_tensor("xvT", (D, T), BF16, kind="ExternalInput")
    wq_d = nc.dram_tensor("wq", (KT, 128, NCH), BF16, kind="ExternalInput")
    wk_d = nc.dram_tensor("wk", (KT, 128, NCH), BF16, kind="ExternalInput")
    wv_d = nc.dram_tensor("wv", (KT, 128, NCH), BF16, kind="ExternalInput")
    bq_d = nc.dram_tensor("bq", (NCH, 1), F32, kind="ExternalInput")
    bk_d = nc.dram_tensor("bk", (NCH, 1), F32, kind="ExternalInput")
    res_d = nc.dram_tensor("resid", (NTILE, 128, NCH), F32, kind="ExternalInput")
    gam_d = nc.dram_tensor("gamma", (1, NCH), F32, kind="ExternalInput")
    bet_d = nc.dram_tensor("beta", (1, NCH), F32, kind="ExternalInput")
    out_d = nc.dram_tensor("out", (NTILE, 128, NCH), F32, kind="ExternalOutput")
    dbg = {}
    if debug_taps:
        import concourse.mybir as _mb
        dbg["qT1"] = nc.dram_tensor("dbg_qT1", (128, S), BF16, kind="ExternalOutput")
        dbg["kT1"] = nc.dram_tensor("dbg_kT1", (128, S), BF16, kind="ExternalOutput")
        dbg["vb1"] = nc.dram_tensor("dbg_vb1", (128, ST, 2, 80), _mb.dt.float8e4, kind="ExternalOutput")
        dbg["op12"] = nc.dram_tensor("dbg_op12", (2, 65, 512), BF16, kind="ExternalOutput")
        dbg["pt12"] = nc.dram_tensor("dbg_pt12", (8, 128, 2, 1024), _mb.dt.float8e4, kind="ExternalOutput")
        dbg["y1"] = nc.dram_tensor("dbg_y1", (128, ST, NCH), F32, kind="ExternalOutput")

    with tile.TileContext(nc) as tc:
        with (
            tc.tile_pool(name="const", bufs=1) as const,
            tc.tile_pool(name="big", bufs=1) as big,
            tc.tile_pool(name="xwin", bufs=3) as xwin,
            tc.tile_pool(name="ptp", bufs=3) as ptp,
            tc.tile_pool(name="otp", bufs=2) as otp,
            tc.tile_pool(name="rpool", bufs=3) as rpool,
            tc.tile_pool(name="small", bufs=6) as small,
            tc.tile_pool(name="auxps", bufs=2, space="PSUM") as auxps,
            tc.tile_pool(name="spps", bufs=2, space="PSUM") as spps,
            tc.tile_pool(name="pvps", bufs=1, space="PSUM") as pvps,
            tc.tile_pool(name="dram", bufs=1, space="DRAM") as dram,
        ):
            identb = const.tile([128, 128], BF16)
            make_identity(nc, identb[:])
            nln16 = const.tile([128, 1], F32)
            nc.vector.memset(nln16[:], -LN32)

            # weights + biases loaded once
            wts = {}
            for nm, w_dram, b_dram in (
                ("q", wq_d, bq_d), ("k", wk_d, bk_d), ("v", wv_d, None),
            ):
                w = const.tile([128, KT, NCH], BF16, tag="w" + nm, name="w" + nm)
                nc.sync.dma_start(w[:], w_dram.ap().rearrange("kt p m -> p kt m"))
                if b_dram is not None:
                    bt = const.tile([NCH, 1], F32, tag="b" + nm, name="b" + nm)
                    nc.sync.dma_start(bt[:], b_dram[:])
                    wts[nm] = (w, bt)
                else:
                    wts[nm] = (w,)

            gam = const.tile([128, NCH], F32)
            nc.sync.dma_start(
                gam[:],
                bass.AP(tensor=gam_d.ap().tensor, offset=0, ap=[[0, 128], [1, NCH]]),
            )
            bet = const.tile([128, NCH], F32)
            nc.sync.dma_start(
                bet[:],
                bass.AP(tensor=bet_d.ap().tensor, offset=0, ap=[[0, 128], [1, NCH]]),
            )

            def load_xwin(x_dram, b, win, pool_tag):
                # [128, KT, 512] bf16 window of x.T: row kt*128+p,
                # col b*S + win*512 + t
                xc = xwin.tile([128, KT, 512], BF16, tag=pool_tag, name="xw")
                nc.sync.dma_start(
                    xc[:],
                    bass.AP(
                        tensor=x_dram.ap().tensor,
                        offset=b * S + win * 512,
                        ap=[[T, 128], [128 * T, KT], [1, 512]],
                    ),
                )
                return xc

            def project_qk(nm, x_dram, b, outT):
                # outT[ch, tok] = W.T @ xT + bias, bf16 out.
                w, bt = wts[nm]
                for win in range(S // 512):
                    xc = load_xwin(x_dram, b, win, "xw")
                    ps = auxps.tile([128, 512], F32, tag="aux", name="pjps")
                    for kt in range(KT):
                        nc.tensor.matmul(
                            ps[:], w[:, kt, :],
                            xc[:, kt, :],
                            start=(kt == 0), stop=(kt == KT - 1),
                        )
                    nc.vector.tensor_scalar_add(
                        outT[:, win * 512 : (win + 1) * 512], ps[:], bt[:]
                    )

            def project_v(x_dram, b, vbuf):
                # v token-major: v[tok, ch] = x @ W; stationary is xT
                # (M=128 tokens), moving is W (N=128 channels); fp8 out.
                # vbuf[128, ST, 2, 80] fp8: per (key tile, head): v in cols
                # 0:64, ones at col 64 (denominator row for the PV matmul).
                w = wts["v"][0]
                for win in range(S // 512):
                    xc = load_xwin(x_dram, b, win, "xw")
                    for s4 in range(4):
                        st = win * 4 + s4
                        ps = auxps.tile([128, 128], F32, tag="aux", name="pvp")
                        for kt in range(KT):
                            nc.tensor.matmul(
                                ps[:],
                                xc[:, kt, s4 * 128 : s4 * 128 + 128],
                                w[:, kt, :],
                                start=(kt == 0), stop=(kt == KT - 1),
                            )
                        # scatter both heads' 64 cols to their 80-aligned slots
                        for h in range(2):
                            nc.vector.tensor_copy(
                                vbuf[:, st, h, 0:64], ps[:, h * 64 : (h + 1) * 64]
                            )

            def attn_chunk(b, tq, qT, kTt, vbuf, y_all, stats):
                t0 = tq * 512
                ops = [
                    pvps.tile([65, 512], F32, tag=f"op{h}", name=f"op{h}")
                    for h in range(2)
                ]
                for kp in range(ST // 2):
                    pt = ptp.tile([128, 2, 1024], FP8, tag="pt", name="pt")
                    tap_pt = debug_taps and b == 1 and tq == 2
                    for i in range(2):
                        st = 2 * kp + i
                        sp = spps.tile([128, 1024], F32, tag="sp", name="sp")
                        for h in range(2):
                            hs = slice(h * 64, (h + 1) * 64)
                            nc.tensor.matmul(
                                sp[:, h * 512 : (h + 1) * 512],
                                kTt[hs, st * 128 : st * 128 + 128],
                                qT[hs, t0 : t0 + 512],
                                start=True, stop=True,
                            )
                        nc.scalar.activation(
                            pt[:, i, :], sp[:], AF.Exp, scale=0.125, bias=nln16[:]
                        )
                    if tap_pt:
                        nc.sync.dma_start(dbg["pt12"].ap()[kp], pt[:])
                    for h in range(2):
                        if dr_pv:
                            nc.tensor.matmul(
                                ops[h][:],
                                vbuf[:, 2 * kp : 2 * kp + 2, h, 0:65],
                                pt[:, :, h * 512 : (h + 1) * 512],
                                start=(kp == 0), stop=(kp == ST // 2 - 1),
                                perf_mode=DR,
                            )
                        else:
                            for i in range(2):
                                nc.tensor.matmul(
                                    ops[h][:],
                                    vbuf[:, 2 * kp + i, h, 0:65],
                                    pt[:, i, h * 512 : (h + 1) * 512],
                                    start=(kp == 0 and i == 0),
                                    stop=(kp == ST // 2 - 1 and i == 1),
                                )
                # psum -> sbuf (bf16), transpose to token-major, divide by
                # denominator (row 64), add residual, partial LN stats
                oTs = []
                for h in range(2):
                    oT = otp.tile([65, 512], BF16, tag=f"oT{h}", name=f"oT{h}")
                    nc.vector.tensor_copy(oT[:], ops[h][:])
                    oTs.append(oT)
                if debug_taps and b == 1 and tq == 2:
                    for h in range(2):
                        nc.sync.dma_start(dbg["op12"].ap()[h], oTs[h][:])
                rt = rpool.tile([128, 4, NCH], F32, tag="rt", name="rt")
                nc.sync.dma_start(
                    rt[:],
                    res_d.ap()[
                        b * ST + tq * 4 : b * ST + tq * 4 + 4
                    ].rearrange("n p m -> p n m"),
                )
                for q4 in range(4):
                    idx = tq * 4 + q4
                    yv = y_all[:, idx, :]
                    rc = small.tile([128, 2], F32, tag="rc", name="rc")
                    for h in range(2):
                        tp = auxps.tile([128, 66], BF16, tag="aux", name="tpo")
                        nc.tensor.transpose(
                            tp[:, 0:65],
                            oTs[h][:, q4 * 128 : (q4 + 1) * 128],
                            identb[0:65, 0:65],
                        )
                        nc.vector.reciprocal(rc[:, h : h + 1], tp[:, 64:65])
                        nc.vector.tensor_scalar(
                            yv[:, h * 64 : (h + 1) * 64], tp[:, 0:64],
                            rc[:, h : h + 1], None, op0=MULT,
                        )
                    nc.vector.tensor_add(yv, yv, rt[:, q4, :])
                    stt = small.tile([128, 6], F32, tag="stt", name="stt")
                    nc.vector.bn_stats(stt[:], yv)
                    mv = small.tile([128, 2], F32, tag="mv", name="mv")
                    nc.vector.bn_aggr(mv[:], stt[:])
                    # stats[idx] = (mean_c, var_c + mean_c^2)
                    nc.vector.tensor_copy(stats[:, idx, 0:1], mv[:, 0:1])
                    sq = small.tile([128, 1], F32, tag="sq", name="sq")
                    nc.vector.tensor_mul(sq[:], mv[:, 0:1], mv[:, 0:1])
                    nc.vector.tensor_add(stats[:, idx, 1:2], mv[:, 1:2], sq[:])

            def one_pass():
                bufs = {}
                def proj_part(b, part):
                    if part == "k":
                        kTt = big.tile([128, S], BF16, tag=f"kT{b}", name=f"kT{b}")
                        project_qk("k", xkT_d, b, kTt)
                        bufs[("k", b)] = kTt
                    elif part == "v":
                        vbuf = big.tile(
                            [128, ST, 2, 80], FP8, tag=f"vb{b}", name=f"vb{b}"
                        )
                        nc.vector.memset(vbuf[:, :, :, 64:65], 1.0)
                        project_v(xvT_d, b, vbuf)
                        bufs[("v", b)] = vbuf
                    else:
                        qT = big.tile([128, S], BF16, tag=f"qT{b}", name=f"qT{b}")
                        project_qk("q", xqT_d, b, qT)
                        bufs[("q", b)] = qT

                ln_state = []
                for b in range(B):
                    y_all = big.tile([128, ST, NCH], F32, tag=f"y{b}", name=f"y{b}")
                    stats = big.tile([128, ST, 2], F32, tag=f"st{b}", name=f"st{b}")
                    ln_state.append((y_all, stats))

                for part in ("k", "v", "q"):
                    proj_part(0, part)
                # batch-0 attention with batch-1 projections interleaved so
                # the in-order PE fills ACT-bound bubbles with proj matmuls
                interleave = ["k", "v", "q", None]
                for tq in range(TQ):
                    attn_chunk(
                        0, tq, bufs[("q", 0)], bufs[("k", 0)], bufs[("v", 0)],
                        ln_state[0][0], ln_state[0][1],
                    )
                    if interleave[tq] is not None:
                        proj_part(1, interleave[tq])
                for tq in range(TQ):
                    attn_chunk(
                        1, tq, bufs[("q", 1)], bufs[("k", 1)], bufs[("v", 1)],
                        ln_state[1][0], ln_state[1][1],
                    )

                if debug_taps:
                    nc.sync.dma_start(dbg["qT1"].ap(), bufs[("q", 1)][:])
                    nc.sync.dma_start(dbg["kT1"].ap(), bufs[("k", 1)][:])
                    nc.sync.dma_start(dbg["vb1"].ap(), bufs[("v", 1)][:])
                    nc.sync.dma_start(dbg["y1"].ap(), ln_state[1][0][:])

                for b in range(B):
                    y_all, stats = ln_state[b]
                    # AllReduce this batch's (mean, E[y^2]) partial sums across
                    # the 8 cores; batch 0's LN tail overlaps batch 1's attention
                    cin = dram.tile([128, ST, 2], F32, tag=f"cin{b}", name=f"cin{b}")
                    cout = dram.tile([128, ST, 2], F32, tag=f"cout{b}", name=f"cout{b}")
                    nc.sync.dma_start(cin[:], stats[:])
                    if with_collective:
                        nc.gpsimd.collective_compute(
                            "AllReduce",
                            mybir.AluOpType.add,
                            replica_groups=[list(range(NCORES))],
                            ins=[cin.opt()],
                            outs=[cout.opt()],
                        )
                    else:  # timeline-sim variant: collective unsupported there
                        nc.sync.dma_start(cout[:], cin[:])
                    ssum = big.tile([128, ST, 2], F32, tag=f"ss{b}", name=f"ss{b}")
                    nc.sync.dma_start(ssum[:], cout[:])

                    # mu = sum(mean_c)/8; var = sum(e2_c)/8 - mu^2; rstd = rsqrt
                    mu = big.tile([128, ST], F32, tag=f"mu{b}", name=f"mu{b}")
                    nc.scalar.mul(mu[:], ssum[:, :, 0], 1.0 / NCORES)
                    e2 = small.tile([128, ST], F32, tag="e2", name="e2")
                    nc.scalar.mul(e2[:], ssum[:, :, 1], 1.0 / NCORES)
                    musq = small.tile([128, ST], F32, tag="musq", name="musq")
                    nc.vector.tensor_mul(musq[:], mu[:], mu[:])
                    av = big.tile([128, ST], F32, tag=f"av{b}", name=f"av{b}")
                    nc.vector.tensor_sub(av[:], e2[:], musq[:])
                    nc.vector.tensor_scalar_add(av[:], av[:], 1e-6)
                    # rstd = rsqrt(a) on DVE only (an ACT Sqrt would thrash the
                    # exp table set mid-kernel): integer-shift exponent seed,
                    # then 5 Newton iterations r' = r*(1.5 - 0.5*a*r^2).
                    rst = big.tile([128, ST], F32, tag=f"rst{b}", name=f"rst{b}")
                    I32 = mybir.dt.int32
                    ei = small.tile([128, ST], I32, tag="ei", name="ei")
                    nc.vector.tensor_scalar(
                        ei[:], av[:].bitcast(I32), 23, None,
                        op0=mybir.AluOpType.logical_shift_right,
                    )
                    nc.vector.tensor_scalar(
                        ei[:], ei[:], -1, 381,
                        op0=mybir.AluOpType.mult, op1=mybir.AluOpType.add,
                    )
                    nc.vector.tensor_scalar(
                        ei[:], ei[:], 1, None,
                        op0=mybir.AluOpType.logical_shift_right,
                    )
                    nc.vector.tensor_scalar(
                        rst[:].bitcast(I32), ei[:], 23, None,
                        op0=mybir.AluOpType.logical_shift_left,
                    )
                    r2 = small.tile([128, ST], F32, tag="r2", name="r2")
                    for _newton in range(5):
                        nc.vector.tensor_mul(r2[:], rst[:], rst[:])
                        nc.vector.tensor_mul(r2[:], r2[:], av[:])
                        nc.vector.tensor_scalar(
                            r2[:], r2[:], -0.5, 1.5,
                            op0=mybir.AluOpType.mult, op1=mybir.AluOpType.add,
                        )
                        nc.vector.tensor_mul(rst[:], rst[:], r2[:])

                    for tq in range(TQ):
                        for q4 in range(4):
                            idx = tq * 4 + q4
                            yv = y_all[:, idx, :]
                            nc.vector.tensor_scalar(
                                yv, yv, mu[:, idx : idx + 1], rst[:, idx : idx + 1],
                                op0=mybir.AluOpType.subtract, op1=mybir.AluOpType.mult,
                            )
                            nc.vector.tensor_mul(yv, yv, gam[:])
                            nc.vector.tensor_add(yv, yv, bet[:])
                        nc.sync.dma_start(
                            out_d.ap()[
                                b * ST + tq * 4 : b * ST + tq * 4 + 4
                            ].rearrange("n p m -> p n m"),
                            y_all[:, tq * 4 : tq * 4 + 4, :],
                        )

            for _rep in range(repeat):
                one_pass()

    nc.compile()
    return nc


def _get_compiled():
    global _COMPILED
    if _COMPILED is None:
        _COMPILED = _build_program()
    return _COMPILED


def _make_in_maps(query, key_, value, Wq, bq, Wk, bk, Wv, bv, ln_gamma, ln_beta):
    import ml_dtypes

    f = np.float32
    bf = ml_dtypes.bfloat16

    q2 = np.ascontiguousarray(query.reshape(T, D), dtype=f)
    xqT = np.ascontiguousarray(q2.T).astype(bf)
    xkT = np.ascontiguousarray(key_.reshape(T, D).T, dtype=f).astype(bf)
    xvT = np.ascontiguousarray(value.reshape(T, D).T, dtype=f).astype(bf)
    bv_f = np.asarray(bv, f)
    in_maps = []
    for c in range(NCORES):
        sl = slice(NCH * c, NCH * (c + 1))
        resid = q2[:, sl] + bv_f[sl][None, :]
        in_maps.append({
            "xqT": xqT,
            "xkT": xkT,
            "xvT": xvT,
            "wq": np.asarray(Wq[:, sl], f).astype(bf).reshape(KT, 128, NCH),
            "wk": np.asarray(Wk[:, sl], f).astype(bf).reshape(KT, 128, NCH),
            "wv": np.asarray(Wv[:, sl], f).astype(bf).reshape(KT, 128, NCH),
            "bq": np.ascontiguousarray(bq[sl], dtype=f).reshape(NCH, 1),
            "bk": np.ascontiguousarray(bk[sl], dtype=f).reshape(NCH, 1),
            "resid": np.ascontiguousarray(resid, dtype=f).reshape(NTILE, 128, NCH),
            "gamma": np.ascontiguousarray(ln_gamma[sl], dtype=f).reshape(1, NCH),
            "beta": np.ascontiguousarray(ln_beta[sl], dtype=f).reshape(1, NCH),
        })
    return in_maps


def kernel(query, key_, value, Wq, bq, Wk, bk, Wv, bv, ln_gamma, ln_beta):
    from concourse import bass_utils

    nc = _get_compiled()
    in_maps = _make_in_maps(
        query, key_, value, Wq, bq, Wk, bk, Wv, bv, ln_gamma, ln_beta
    )
    res = bass_utils.run_bass_kernel_spmd(nc, in_maps, core_ids=list(range(NCORES)))
    slices = [res.results[c]["out"].reshape(T, NCH) for c in range(NCORES)]
    out = np.concatenate(slices, axis=1)
    return out.reshape(B, S, D)
